# revision 15
# baseline (speedup 1.0000x reference)
"""Trainium2 Bass kernel for nn_CascadeEncoderFirst (dense CNN cascade).

Sharding: 8 cores = 4 samples x 2 row-halves. The row-half split is made
SPMD-uniform by vertically flipping the sample (and all 3x3 kernels /
low-pass operators) for the h=1 cores, so every core computes rows
[0,128) (+2 halo rows) of its possibly-flipped sample. Instance-norm
statistics are completed with tiny pair-wise AllReduces.

Key rewrites vs the reference:
  - FFT low-pass  ->  Y = Arext X Ar^T - Aiext X Ai^T  (precomputed DFT
    band matrices; Ar symmetric, Ai antisymmetric, so the step-2 right
    operands are [Ar | Ai]).
  - grouped conv w_con + 1x1 w_u1 (both linear, back-to-back) folded
    into one dense 3x3 conv W_comb (128 -> 64).
All matmuls run in fp16 (full PE rate) with fp32 PSUM accumulation;
statistics and normalization are fp32.
"""

import numpy as np

C = 64
H = 256
PH = 258                 # padded row width (1px zero border each side)
EXT = 130                # rows computed per core (128 out + 2 halo)
NT_EXT = 65              # 2-row psum tiles for ext-row stages
NT_OUT = 64              # 2-row psum tiles for out-row stages
NPIX = float(H * H)      # instance-norm population size
EPS = 1e-5
NEG = 0.01
TAPS = [(dy, dx) for dy in (-1, 0, 1) for dx in (-1, 0, 1)]
PADF = PH * 132 + 8      # free size of padded 132-row sbuf images

_CACHE = {}


# ----------------------------------------------------------------------
# host-side constant preparation
# ----------------------------------------------------------------------

def lowpass_mats():
    """A = F^-1 diag(ifftshift(mask)) F for one axis (float64)."""
    hl = int(H * 0.3 // 2)
    hc = H // 2
    m = np.zeros(H, bool)
    m[hc - hl:hc + hl] = True
    F = np.fft.fft(np.eye(H))
    Finv = np.fft.ifft(np.eye(H))
    A = Finv @ np.diag(np.fft.ifftshift(m).astype(np.float64)) @ F
    return np.ascontiguousarray(A.real), np.ascontiguousarray(A.imag)


def host_prep(inputs):
    """Build per-core device input dicts (8 entries, core = 2*s + h)."""
    f16 = np.float16
    f32 = np.float32

    x = np.asarray(inputs['x'], f32)              # [4, 3, 256, 256]
    w_proj = np.asarray(inputs['w_proj'], f32)    # [64, 3, 3, 3]
    b_proj = np.asarray(inputs['b_proj'], f32)
    w_a1 = np.asarray(inputs['w_a1'], f32)        # [16, 64, 1, 1]
    w_a2 = np.asarray(inputs['w_a2'], f32)        # [64, 16, 1, 1]
    w_ref = np.asarray(inputs['w_ref'], f32)      # [64, 1, 3, 3]
    b_ref = np.asarray(inputs['b_ref'], f32)
    w_fuse = np.asarray(inputs['w_fuse'], f32)    # [64, 64, 1, 1]
    b_fuse = np.asarray(inputs['b_fuse'], f32)
    w_con = np.asarray(inputs['w_con'], f32)      # [512, 1, 3, 3]
    w_u1 = np.asarray(inputs['w_u1'], f32)        # [64, 512, 1, 1]
    b_u1 = np.asarray(inputs['b_u1'], f32)
    w_u2 = np.asarray(inputs['w_u2'], f32)        # [64, 64, 3, 3]
    b_u2 = np.asarray(inputs['b_u2'], f32)
    w_sc = np.asarray(inputs['w_sc'], f32)        # [64, 128, 1, 1]
    b_sc = np.asarray(inputs['b_sc'], f32)

    # fold grouped conv + 1x1 into dense 3x3: W_comb[o,g,ky,kx]
    wc4 = w_con.reshape(2 * C, 4, 3, 3)
    wu = w_u1[:, :, 0, 0].reshape(C, 2 * C, 4)
    W_comb = np.einsum('ogk,gkyx->ogyx', wu, wc4)

    Ar, Ai = lowpass_mats()
    P = np.eye(H)[::-1]

    def pack9(w):
        # dense 3x3 [O=64, I, 3, 3] -> lhsT [I, 9*64] in TAPS order
        return np.concatenate(
            [np.ascontiguousarray(w[:, :, dy + 1, dx + 1].T)
             for (dy, dx) in TAPS], axis=1)

    def pack_pairs_dense(w):
        # pairs (dy,-1)+(dy,0) stacked -> [2I, 3*64]; singles (dy,+1) -> [I, 3*64]
        Iin = w.shape[1]
        wp = np.zeros((2 * Iin, 3 * C), f32)
        ws = np.zeros((Iin, 3 * C), f32)
        for pi, dy in enumerate((-1, 0, 1)):
            wp[0:Iin, pi * C:(pi + 1) * C] = w[:, :, dy + 1, 0].T
            wp[Iin:2 * Iin, pi * C:(pi + 1) * C] = w[:, :, dy + 1, 1].T
            ws[:, pi * C:(pi + 1) * C] = w[:, :, dy + 1, 2].T
        return wp, ws

    def ref_dense(wr):
        # depthwise [64,1,3,3] -> dense-diagonal [64,64,3,3]
        wd = np.zeros((C, C, 3, 3), f32)
        for o in range(C):
            wd[o, o] = wr[o, 0]
        return wd

    per_core = []
    for s in range(4):
        for h in (0, 1):
            flip = (h == 1)

            def fl(w):     # flip 3x3 kernels vertically
                return w[:, :, ::-1, :] if flip else w

            xs = x[s]
            if flip:
                xs = xs[:, ::-1, :]
            xpad = np.zeros((3, PH, PH), f16)
            xpad[:, 1:257, 1:257] = xs.astype(f16)

            if flip:
                Arf, Aif = P @ Ar @ P, P @ Ai @ P
            else:
                Arf, Aif = Ar, Ai
            # step1 rhs: [256, 260] = [Arf_ext^T | Aif_ext^T]
            lp1 = np.concatenate([Arf[0:EXT, :].T, Aif[0:EXT, :].T],
                                 axis=1).astype(f16)
            # step2 rhs: [256, 512] = [Ar | Ai] (uses Ar^T=Ar, -Ai^T=Ai)
            lp2 = np.concatenate([Ar, Ai], axis=1).astype(f16)

            wrefp, wrefs = pack_pairs_dense(ref_dense(fl(w_ref)))
            wu2p, wu2s = pack_pairs_dense(fl(w_u2))

            wproj_t = np.zeros((27, C), f32)
            wpf = fl(w_proj)
            for t, (dy, dx) in enumerate(TAPS):
                wproj_t[3 * t:3 * t + 3, :] = wpf[:, :, dy + 1, dx + 1].T

            d = {
                'xpad': np.ascontiguousarray(xpad),
                'wproj': wproj_t.astype(f16),
                'bproj': b_proj.reshape(C, 1).astype(f32),
                'wa1t': np.ascontiguousarray(w_a1[:, :, 0, 0].T).astype(f16),
                'wa2t': np.ascontiguousarray(w_a2[:, :, 0, 0].T).astype(f16),
                'wrefp': wrefp.astype(f16),
                'wrefs': wrefs.astype(f16),
                'bref': b_ref.reshape(C, 1).astype(f32),
                'wfuse': np.ascontiguousarray(w_fuse[:, :, 0, 0].T).astype(f16),
                'bfuse': b_fuse.reshape(C, 1).astype(f32),
                'wcomb': pack9(fl(W_comb)).astype(f16),
                'bu1': b_u1.reshape(C, 1).astype(f32),
                'wu2p': wu2p.astype(f16),
                'wu2s': wu2s.astype(f16),
                'bu2': b_u2.reshape(C, 1).astype(f32),
                'wsc': np.ascontiguousarray(w_sc[:, :, 0, 0].T).astype(f16),
                'bsc': b_sc.reshape(C, 1).astype(f32),
                'lp1': np.ascontiguousarray(lp1),
                'lp2': np.ascontiguousarray(lp2),
            }
            per_core.append(d)
    return per_core


# ----------------------------------------------------------------------
# walrus workarounds
# ----------------------------------------------------------------------

def _split_multi_waits(bir_json):
    """This walrus accepts at most one semaphore wait per instruction,
    but Tile's scheduler attaches one wait per producer proc. Split the
    extras onto NoOps inserted immediately before the instruction (same
    engine) — semantically identical, and deadlock-free because Tile's
    list scheduling keeps every engine stream in global topological
    order."""
    import json
    j = json.loads(bir_json)
    ctr = [0]

    def fix(bk):
        if isinstance(bk, dict):
            insts = bk.get('instructions')
            if isinstance(insts, list):
                out = []
                for inst in insts:
                    if isinstance(inst, dict):
                        si = inst.get('sync_info')
                        ow = (si or {}).get('on_wait') or []
                        if len(ow) > 1:
                            for w in ow[:-1]:
                                ctr[0] += 1
                                out.append({
                                    "debug": inst.get("debug", 0),
                                    "engine": inst.get("engine") or "SP",
                                    "ins": [], "outs": [],
                                    "name": f"I-ws{ctr[0]}",
                                    "opcode": "NoOp",
                                    "sync_info": {"on_update": [],
                                                  "on_wait": [w]},
                                })
                            si['on_wait'] = [ow[-1]]
                    out.append(inst)
                bk['instructions'] = out
            for k, v in bk.items():
                if k != 'instructions' and isinstance(v, (list, dict)):
                    fix(v)
        elif isinstance(bk, list):
            for e in bk:
                fix(e)

    for fn in j['functions']:
        fix(fn['blocks'])
    return json.dumps(j).encode()


def _install_compile_patch():
    import concourse.bass_utils as bu
    import concourse.bass2jax as b2j
    if getattr(bu, '_split_waits_patched', False):
        return
    orig = bu.compile_bir_kernel

    def patched(bir_json, tmpdir, neff_name="file.neff"):
        return orig(_split_multi_waits(bir_json), tmpdir, neff_name)

    bu.compile_bir_kernel = patched
    b2j.compile_bir_kernel = patched
    bu._split_waits_patched = True


# ----------------------------------------------------------------------
# tile-drain workaround
# ----------------------------------------------------------------------

def _install_tile_patch():
    """walrus accepts at most ONE sem wait per sync-engine instruction;
    this concourse version's TileContext final drain piles every
    outstanding proc's wait onto a single Drain. Split them."""
    import concourse.mybir as mybir
    import concourse.tile as tile_mod

    if getattr(tile_mod, '_cascade_drain_patched', False):
        return

    def _patched(self, tick_clock, wait_clock):
        nc = self.nc
        drain_inst = nc.sync.drain()
        wait_clock.add_sem_waits(
            drain_inst.ins, tile_mod.ScopedClock({None: tick_clock.global_clock})
        )
        si = drain_inst.ins.sync_info
        if si is not None:
            waits = list(si.on_wait)
            if len(waits) > 1:
                si.on_wait = waits[:1]
                for w in waits[1:]:
                    n = nc.sync.nop(nofuse=True)
                    n.ins.sync_info = mybir.SyncInfo(on_wait=[w], on_update=[])
        nc.all_engine_barrier()
        assert self.sems is not None
        popped = nc._tile_sem_poison_stack.pop()
        assert popped is self._sem_poison
        nc.clear_and_free_semaphores(list(self.sems.allocated().values()))
        nc.all_engine_barrier()

    tile_mod.TileContext._drain_and_barrier = _patched
    tile_mod._cascade_drain_patched = True


# ----------------------------------------------------------------------
# device program
# ----------------------------------------------------------------------

def _build_nc():
    _install_tile_patch()
    _install_compile_patch()
    from contextlib import ExitStack

    import concourse.bass as bass
    import concourse.mybir as mybir
    from concourse.tile import TileContext

    f16 = mybir.dt.float16
    f32 = mybir.dt.float32
    AF = mybir.ActivationFunctionType
    ALU = mybir.AluOpType
    AX = mybir.AxisListType

    nc = bass.Bass(trn_type="TRN2", num_devices=8)

    def din(name, shape, dt=f16):
        return nc.dram_tensor(name, shape, dt, kind="ExternalInput")

    xpad_d = din('xpad', [3, PH, PH])
    wproj_d = din('wproj', [27, C])
    bproj_d = din('bproj', [C, 1], f32)
    wa1t_d = din('wa1t', [C, 16])
    wa2t_d = din('wa2t', [16, C])
    wrefp_d = din('wrefp', [2 * C, 3 * C])
    wrefs_d = din('wrefs', [C, 3 * C])
    bref_d = din('bref', [C, 1], f32)
    wfuse_d = din('wfuse', [C, C])
    bfuse_d = din('bfuse', [C, 1], f32)
    wcomb_d = din('wcomb', [2 * C, 9 * C])
    bu1_d = din('bu1', [C, 1], f32)
    wu2p_d = din('wu2p', [2 * C, 3 * C])
    wu2s_d = din('wu2s', [C, 3 * C])
    bu2_d = din('bu2', [C, 1], f32)
    wsc_d = din('wsc', [2 * C, C])
    bsc_d = din('bsc', [C, 1], f32)
    lp1_d = din('lp1', [256, 260])
    lp2_d = din('lp2', [256, 512])

    feat_o = nc.dram_tensor('feat', [C, 128, 256], f32, kind="ExternalOutput")
    pooled_o = nc.dram_tensor('pooled', [C, 64, 128], f32, kind="ExternalOutput")

    xp_pad = nc.dram_tensor('xp_pad', [C, PH, PH], f16, kind="Internal")
    low0_d = nc.dram_tensor('low0', [C, EXT, 256], f16, kind="Internal")
    xc_pad = nc.dram_tensor('xc_pad', [2 * C, PH, PH], f16, kind="Internal")
    sc_d = nc.dram_tensor('sc_d', [C, 128, 256], f32, kind="Internal")

    cc_in = [nc.dram_tensor(f'cc{i}_in', [1, 256], f32, kind="Internal")
             for i in range(3)]
    cc_out = [nc.dram_tensor(f'cc{i}_out', [1, 256], f32, kind="Internal")
              for i in range(3)]
    RG = [[0, 1], [2, 3], [4, 5], [6, 7]]

    xp_flat = xp_pad.rearrange("c h w -> c (h w)")
    xc_flat = xc_pad.rearrange("c h w -> c (h w)")

    def ap2d(tile, parts, off, rows, cols=256):
        return (tile[0:parts, off:off + rows * PH]
                .rearrange("p (r c) -> p r c", c=PH)[:, :, 0:cols])

    import os
    REP = int(os.environ.get('CASCADE_REP', '1'))
    with TileContext(nc) as tc:
      for _rep in range(REP):
        es = ExitStack()
        with es:
            wpool = es.enter_context(tc.tile_pool(name="weights", bufs=1))
            spool = es.enter_context(tc.tile_pool(name="stats", bufs=1))

            # ---- load constants ------------------------------------
            def wload(dram, shape, dt=f16):
                t = wpool.tile(list(shape), dt, tag=dram.name, name=dram.name)
                nc.sync.dma_start(t[0:shape[0], 0:shape[1]], dram[:, :])
                return t

            wproj_sb = wload(wproj_d, (27, C))
            bproj_sb = wload(bproj_d, (C, 1), f32)
            wa1t_sb = wload(wa1t_d, (C, 16))
            wa2t_sb = wload(wa2t_d, (16, C))
            wrefp_sb = wload(wrefp_d, (2 * C, 3 * C))
            wrefs_sb = wload(wrefs_d, (C, 3 * C))
            bref_sb = wload(bref_d, (C, 1), f32)
            wfuse_sb = wload(wfuse_d, (C, C))
            bfuse_sb = wload(bfuse_d, (C, 1), f32)
            wcomb_sb = wload(wcomb_d, (2 * C, 9 * C))
            bu1_sb = wload(bu1_d, (C, 1), f32)
            wu2p_sb = wload(wu2p_d, (2 * C, 3 * C))
            wu2s_sb = wload(wu2s_d, (C, 3 * C))
            bu2_sb = wload(bu2_d, (C, 1), f32)
            wsc_sb = wload(wsc_d, (2 * C, C))
            bsc_sb = wload(bsc_d, (C, 1), f32)

            lp1a = wpool.tile([128, 260], f16, tag="lp1a", name="lp1a")
            lp1b = wpool.tile([128, 260], f16, tag="lp1b", name="lp1b")
            nc.sync.dma_start(lp1a[:, :], lp1_d[0:128, :])
            nc.sync.dma_start(lp1b[:, :], lp1_d[128:256, :])
            lp2a = wpool.tile([128, 512], f16, tag="lp2a", name="lp2a")
            lp2b = wpool.tile([128, 512], f16, tag="lp2b", name="lp2b")
            nc.sync.dma_start(lp2a[:, :], lp2_d[0:128, :])
            nc.sync.dma_start(lp2b[:, :], lp2_d[128:256, :])

            # zero tile for DRAM border strips
            zrow = wpool.tile([2 * C, 520], f16, tag="zrow", name="zrow")
            nc.vector.memset(zrow[:, :], 0.0)

            def zero_borders(flat, parts):
                nc.sync.dma_start(flat[0:parts, 0:259], zrow[0:parts, 0:259])
                nc.sync.dma_start(flat[0:parts, 257 * PH:258 * PH],
                                  zrow[0:parts, 0:PH])
                nc.sync.dma_start(
                    flat[0:parts, 515:515 + 256 * PH].rearrange(
                        "c (r k) -> c r k", k=PH)[:, :, 0:2],
                    zrow[0:parts, 0:512].rearrange("c (r k) -> c r k", k=2))

            zero_borders(xp_flat, C)
            zero_borders(xc_flat, 2 * C)

            # ---- stats tiles ---------------------------------------
            att_cols = spool.tile([C, 128], f32, tag="att_cols", name="att_cols")
            ref_cols = spool.tile([C, NT_EXT], f32, tag="ref_cols", name="ref_cols")
            fuse_cols = spool.tile([C, NT_EXT], f32, tag="fuse_cols", name="fuse_cols")
            u1_cols = spool.tile([C, NT_EXT], f32, tag="u1_cols", name="u1_cols")
            u2_cols = spool.tile([C, NT_OUT], f32, tag="u2_cols", name="u2_cols")
            sq_cols = spool.tile([C, 32], f32, tag="sq_cols", name="sq_cols")
            trash = spool.tile([C, 4096], f16, tag="trash", name="trash")
            epsb = spool.tile([C, 1], f32, tag="epsb", name="epsb")
            nc.vector.memset(epsb[:, :], float(EPS))

            def sumsq(src, base):
                for i in range(8):
                    nc.scalar.activation(
                        trash[0:C, 0:4096], src[0:C, i * 4096:(i + 1) * 4096],
                        AF.Square, bias=0.0, scale=1.0,
                        accum_out=sq_cols[0:C, base + i:base + i + 1])

            def stat_pack(cc_in_d, aps):
                for j, ap in enumerate(aps):
                    red = spool.tile([C, 1], f32, tag=f"red_{cc_in_d.name}_{j}",
                                     name=f"red_{cc_in_d.name}_{j}")
                    nc.vector.tensor_reduce(red[0:C, 0:1], ap, axis=AX.X,
                                            op=ALU.add)
                    nc.sync.dma_start(cc_in_d[0:1, j * C:(j + 1) * C],
                                      red[0:C, 0:1])

            def norm_coeffs(cc_out_d, idx, tagp):
                s = spool.tile([C, 1], f32, tag=f"{tagp}_s", name=f"{tagp}_s")
                s2 = spool.tile([C, 1], f32, tag=f"{tagp}_s2", name=f"{tagp}_s2")
                nc.sync.dma_start(s[0:C, 0:1],
                                  cc_out_d[0:1, idx * C:(idx + 1) * C])
                nc.sync.dma_start(s2[0:C, 0:1],
                                  cc_out_d[0:1, (idx + 1) * C:(idx + 2) * C])
                mu = spool.tile([C, 1], f32, tag=f"{tagp}_mu", name=f"{tagp}_mu")
                ex2 = spool.tile([C, 1], f32, tag=f"{tagp}_ex2", name=f"{tagp}_ex2")
                nc.scalar.activation(mu[0:C, 0:1], s[0:C, 0:1], AF.Copy,
                                     bias=0.0, scale=float(1.0 / NPIX))
                nc.scalar.activation(ex2[0:C, 0:1], s2[0:C, 0:1], AF.Copy,
                                     bias=0.0, scale=float(1.0 / NPIX))
                var = spool.tile([C, 1], f32, tag=f"{tagp}_var", name=f"{tagp}_var")
                nc.vector.tensor_tensor(var[0:C, 0:1], mu[0:C, 0:1],
                                        mu[0:C, 0:1], op=ALU.mult)
                nc.vector.tensor_sub(var[0:C, 0:1], ex2[0:C, 0:1], var[0:C, 0:1])
                std = spool.tile([C, 1], f32, tag=f"{tagp}_std", name=f"{tagp}_std")
                nc.scalar.activation(std[0:C, 0:1], var[0:C, 0:1], AF.Sqrt,
                                     bias=epsb[0:C, 0:1], scale=1.0)
                inv = spool.tile([C, 1], f32, tag=f"{tagp}_inv", name=f"{tagp}_inv")
                nc.vector.reciprocal(inv[0:C, 0:1], std[0:C, 0:1])
                nb = spool.tile([C, 1], f32, tag=f"{tagp}_nb", name=f"{tagp}_nb")
                nc.vector.tensor_tensor(nb[0:C, 0:1], mu[0:C, 0:1],
                                        inv[0:C, 0:1], op=ALU.mult)
                nc.vector.tensor_scalar_mul(nb[0:C, 0:1], nb[0:C, 0:1], -1.0)
                return inv, nb

            # ================= PHASE P: proj conv ===================
            with tc.tile_pool(name="xreppool", bufs=1) as xreppool, \
                 tc.tile_pool(name="projps", bufs=4, space="PSUM") as projps, \
                 tc.tile_pool(name="projout", bufs=6) as projout:
                xrep = xreppool.tile([27, 66568], f16, tag="xrep", name="xrep")
                xpad_flat = xpad_d.rearrange("c h w -> c (h w)")
                for t, (dy, dx) in enumerate(TAPS):
                    delta = (dy + 1) * PH + (dx + 1)
                    nc.sync.dma_start(xrep[3 * t:3 * t + 3, 0:66046],
                                      xpad_flat[0:3, delta:delta + 66046])
                for t in range(128):
                    y0 = 2 * t
                    ps = projps.tile([C, 512], f32, tag="ps", name="projpst")
                    psv = ps[0:C, 0:512].rearrange("p (r c) -> p r c", c=256)
                    nc.tensor.matmul(psv, wproj_sb[0:27, 0:C],
                                     ap2d(xrep, 27, y0 * PH, 2),
                                     start=True, stop=True)
                    ot = projout.tile([C, 512], f16, tag="ot", name="projott")
                    nc.scalar.activation(ot[0:C, 0:512], ps[0:C, 0:512],
                                         AF.Identity, bias=bproj_sb[0:C, 0:1],
                                         scale=1.0,
                                         accum_out=att_cols[0:C, t:t + 1])
                    nc.sync.dma_start(xp_pad[0:C, 1 + y0:3 + y0, 1:257],
                                      ot[0:C, 0:512].rearrange(
                                          "p (r c) -> p r c", c=256))

            # ================= attention MLP ========================
            att = spool.tile([C, 1], f32, tag="att", name="att")
            attrep = spool.tile([2 * C, 1], f32, tag="attrep", name="attrep")
            m16 = spool.tile([C, 1], f16, tag="m16", name="m16")
            a1h = spool.tile([16, 1], f16, tag="a1h", name="a1h")
            xsum = spool.tile([C, 1], f32, tag="xsum", name="xsum")
            with tc.tile_pool(name="attps", bufs=2, space="PSUM") as attps:
                nc.vector.tensor_reduce(xsum[0:C, 0:1], att_cols[0:C, 0:128],
                                        axis=AX.X, op=ALU.add)
                nc.scalar.activation(m16[0:C, 0:1], xsum[0:C, 0:1], AF.Copy,
                                     bias=0.0, scale=float(1.0 / NPIX))
                pa = attps.tile([16, 1], f32, tag="pa", name="pa")
                nc.tensor.matmul(pa[0:16, 0:1], wa1t_sb[0:C, 0:16],
                                 m16[0:C, 0:1], start=True, stop=True)
                nc.scalar.activation(a1h[0:16, 0:1], pa[0:16, 0:1], AF.Lrelu,
                                     bias=0.0, scale=1.0, alpha=NEG)
                pb = attps.tile([C, 1], f32, tag="pb", name="pb")
                nc.tensor.matmul(pb[0:C, 0:1], wa2t_sb[0:16, 0:C],
                                 a1h[0:16, 0:1], start=True, stop=True)
                nc.scalar.activation(att[0:C, 0:1], pb[0:C, 0:1], AF.Sigmoid,
                                     bias=0.0, scale=1.0)
            nc.sync.dma_start(attrep[0:C, 0:1], att[0:C, 0:1])
            nc.sync.dma_start(attrep[C:2 * C, 0:1], att[0:C, 0:1])
            wrefp_s = wpool.tile([2 * C, 3 * C], f16, tag="wrefp_s", name="wrefp_s")
            wrefs_s = wpool.tile([C, 3 * C], f16, tag="wrefs_s", name="wrefs_s")
            nc.scalar.activation(wrefp_s[:, :], wrefp_sb[:, :], AF.Copy,
                                 bias=0.0, scale=attrep[0:2 * C, 0:1])
            nc.scalar.activation(wrefs_s[:, :], wrefs_sb[:, :], AF.Copy,
                                 bias=0.0, scale=att[0:C, 0:1])

            # ========== PHASE L+R: lowpass & refine (overlap) =======
            ref_es = ExitStack()
            refpool = ref_es.enter_context(tc.tile_pool(name="refpool", bufs=1))
            ref_pre = refpool.tile([C, NT_EXT * 512], f16, tag="ref_pre",
                                   name="ref_pre")

            with tc.tile_pool(name="lrps", bufs=2, space="PSUM") as lrps, \
                 tc.tile_pool(name="lowsb", bufs=4) as lowsb, \
                 tc.tile_pool(name="xpd2p", bufs=1, side="right") as xpd2p:

                # refine: dense-diagonal 3x3, pair-packed
                xpd2 = xpd2p.tile([2 * C, PADF], f16, tag="xpd2", name="xpd2")
                nc.sync.dma_start(xpd2[0:C, 0:132 * PH],
                                  xp_flat[0:C, 0:132 * PH])
                nc.sync.dma_start(xpd2[C:2 * C, 0:132 * PH],
                                  xp_flat[0:C, 1:132 * PH + 1])
                for t in range(NT_EXT):
                    y0 = 2 * t
                    ps = lrps.tile([C, 512], f32, tag="refps", name="refpst",
                                   bufs=3)
                    psv = ps[0:C, 0:512].rearrange("p (r c) -> p r c", c=256)
                    mms = []
                    for pi, dy in enumerate((-1, 0, 1)):
                        mms.append((wrefp_s[0:2 * C, pi * C:(pi + 1) * C],
                                    ap2d(xpd2, 2 * C, (y0 + dy + 1) * PH, 2)))
                    for si, dy in enumerate((-1, 0, 1)):
                        mms.append((wrefs_s[0:C, si * C:(si + 1) * C],
                                    ap2d(xpd2, C, (y0 + dy + 1) * PH + 2, 2)))
                    for i, (lh, rh) in enumerate(mms):
                        nc.tensor.matmul(psv, lh, rh, start=(i == 0),
                                         stop=(i == len(mms) - 1))
                    nc.scalar.activation(
                        ref_pre[0:C, t * 512:(t + 1) * 512], ps[0:C, 0:512],
                        AF.Identity, bias=bref_sb[0:C, 0:1], scale=1.0,
                        accum_out=(ref_cols[0:C, t:t + 1] if t < 64 else
                                   ref_cols[0:C, 64:65]))

                # lowpass per channel
                for c in range(C):
                    xk0 = lowsb.tile([128, 256], f16, tag="xk0", name="xk0")
                    xk1 = lowsb.tile([128, 256], f16, tag="xk1", name="xk1")
                    nc.sync.dma_start(xk0[:, :], xp_pad[c:c + 1, 1:129, 1:257])
                    nc.sync.dma_start(xk1[:, :], xp_pad[c:c + 1, 129:257, 1:257])
                    t1 = []
                    for m in range(2):
                        ps1 = lrps.tile([128, 260], f32, tag="ps1",
                                        name="ps1t", bufs=2)
                        nc.tensor.matmul(ps1[:, :], xk0[:, m * 128:(m + 1) * 128],
                                         lp1a[:, :], start=True, stop=False)
                        nc.tensor.matmul(ps1[:, :], xk1[:, m * 128:(m + 1) * 128],
                                         lp1b[:, :], start=False, stop=True)
                        t1m = lowsb.tile([128, 260], f16, tag=f"t1_{m}",
                                         name=f"t1_{m}")
                        nc.vector.tensor_copy(t1m[:, :], ps1[:, :])
                        t1.append(t1m)
                    ps2 = lrps.tile([128, 256], f32, tag="ps2", name="ps2t",
                                    bufs=2)
                    nc.tensor.matmul(ps2[:, :], t1[0][:, 0:128], lp2a[:, 0:256],
                                     start=True, stop=False)
                    nc.tensor.matmul(ps2[:, :], t1[1][:, 0:128], lp2b[:, 0:256],
                                     start=False, stop=False)
                    nc.tensor.matmul(ps2[:, :], t1[0][:, 130:258],
                                     lp2a[:, 256:512], start=False, stop=False)
                    nc.tensor.matmul(ps2[:, :], t1[1][:, 130:258],
                                     lp2b[:, 256:512], start=False, stop=True)
                    lo = lowsb.tile([128, 256], f16, tag="lo", name="lo")
                    nc.scalar.activation(lo[:, :], ps2[:, :], AF.Copy,
                                         bias=0.0, scale=1.0)
                    nc.sync.dma_start(low0_d[c:c + 1, 0:128, :], lo[:, :])
                    ps2t = lrps.tile([2, 256], f32, tag="ps2tl", name="ps2tlt",
                                     bufs=1)
                    nc.tensor.matmul(ps2t[:, :], t1[0][:, 128:130],
                                     lp2a[:, 0:256], start=True, stop=False)
                    nc.tensor.matmul(ps2t[:, :], t1[1][:, 128:130],
                                     lp2b[:, 0:256], start=False, stop=False)
                    nc.tensor.matmul(ps2t[:, :], t1[0][:, 258:260],
                                     lp2a[:, 256:512], start=False, stop=False)
                    nc.tensor.matmul(ps2t[:, :], t1[1][:, 258:260],
                                     lp2b[:, 256:512], start=False, stop=True)
                    lot = lowsb.tile([2, 256], f16, tag="lot", name="lot")
                    nc.scalar.activation(lot[:, :], ps2t[:, :], AF.Copy,
                                         bias=0.0, scale=1.0)
                    nc.sync.dma_start(low0_d[c:c + 1, 128:130, :], lot[:, :])

            # ================= fuse conv ============================
            fuse_es = ExitStack()
            fusepool = fuse_es.enter_context(tc.tile_pool(name="fusepool",
                                                          bufs=1))
            fuse_pre = fusepool.tile([C, NT_EXT * 512], f16, tag="fuse_pre",
                                     name="fuse_pre")
            with tc.tile_pool(name="fuseps", bufs=4, space="PSUM") as fuseps, \
                 tc.tile_pool(name="fusein", bufs=4) as fuseinp:
                low_flat = low0_d.rearrange("c h w -> c (h w)")
                for t in range(NT_EXT):
                    fin = fuseinp.tile([C, 512], f16, tag="fin", name="fin")
                    nc.sync.dma_start(fin[0:C, 0:512],
                                      low_flat[0:C, t * 512:(t + 1) * 512])
                    ps = fuseps.tile([C, 512], f32, tag="ps", name="fusepst")
                    nc.tensor.matmul(ps[0:C, 0:512], wfuse_sb[0:C, 0:C],
                                     fin[0:C, 0:512], start=True, stop=True)
                    nc.scalar.activation(
                        fuse_pre[0:C, t * 512:(t + 1) * 512], ps[0:C, 0:512],
                        AF.Identity, bias=bfuse_sb[0:C, 0:1], scale=1.0,
                        accum_out=(fuse_cols[0:C, t:t + 1] if t < 64 else
                                   fuse_cols[0:C, 64:65]))

            # ---- AR1: refine + fuse stats --------------------------
            sumsq(ref_pre, 0)
            sumsq(fuse_pre, 8)
            stat_pack(cc_in[0], [ref_cols[0:C, 0:64], sq_cols[0:C, 0:8],
                                 fuse_cols[0:C, 0:64], sq_cols[0:C, 8:16]])
            nc.gpsimd.collective_compute(
                "AllReduce", ALU.add, replica_groups=RG,
                ins=[cc_in[0][0:1, 0:256]], outs=[cc_out[0][0:1, 0:256]])
            inv_r, nb_r = norm_coeffs(cc_out[0], 0, "ref")
            inv_f, nb_f = norm_coeffs(cc_out[0], 2, "fus")

            # ---- normalize+lrelu -> xc_pad (DRAM) ------------------
            with tc.tile_pool(name="nstage", bufs=4) as nstage:
                CH = 13 * 256  # 13 rows per chunk, 10 chunks = 130 rows
                for src, invt, nbt, pbase in ((ref_pre, inv_r, nb_r, 0),
                                              (fuse_pre, inv_f, nb_f, C)):
                    for k in range(10):
                        st = nstage.tile([C, CH], f16, tag="st", name="nst")
                        nc.scalar.activation(
                            st[0:C, 0:CH], src[0:C, k * CH:(k + 1) * CH],
                            AF.Lrelu, bias=nbt[0:C, 0:1],
                            scale=invt[0:C, 0:1], alpha=NEG)
                        nc.sync.dma_start(
                            xc_pad[pbase:pbase + C,
                                   1 + 13 * k:1 + 13 * (k + 1), 1:257],
                            st[0:C, 0:CH].rearrange("p (r c) -> p r c", c=256))
            fuse_es.close()
            ref_es.close()

            # ========== ucomb + sc ==================================
            u1_es = ExitStack()
            u1pool = u1_es.enter_context(tc.tile_pool(name="u1pool", bufs=1))
            u1_pre = u1pool.tile([C, NT_EXT * 512], f16, tag="u1_pre",
                                 name="u1_pre")
            with tc.tile_pool(name="xc2p", bufs=1, side="right") as xc2p, \
                 tc.tile_pool(name="ucps", bufs=4, space="PSUM") as ucps, \
                 tc.tile_pool(name="scps", bufs=2, space="PSUM") as scps, \
                 tc.tile_pool(name="scout", bufs=4) as scoutp:
                xc2 = xc2p.tile([2 * C, PADF], f16, tag="xc2", name="xc2")
                nc.sync.dma_start(xc2[0:2 * C, 0:132 * PH],
                                  xc_flat[0:2 * C, 0:132 * PH])
                for t in range(NT_EXT):
                    y0 = 2 * t
                    ps = ucps.tile([C, 512], f32, tag="ps", name="ucpst")
                    psv = ps[0:C, 0:512].rearrange("p (r c) -> p r c", c=256)
                    for i, (dy, dx) in enumerate(TAPS):
                        rh = ap2d(xc2, 2 * C, (y0 + dy + 1) * PH + (dx + 1), 2)
                        nc.tensor.matmul(psv,
                                         wcomb_sb[0:2 * C, i * C:(i + 1) * C],
                                         rh, start=(i == 0), stop=(i == 8))
                    nc.scalar.activation(
                        u1_pre[0:C, t * 512:(t + 1) * 512], ps[0:C, 0:512],
                        AF.Identity, bias=bu1_sb[0:C, 0:1], scale=1.0,
                        accum_out=(u1_cols[0:C, t:t + 1] if t < 64 else
                                   u1_cols[0:C, 64:65]))
                for t in range(NT_OUT):
                    y0 = 2 * t
                    ps = scps.tile([C, 512], f32, tag="ps", name="scpst")
                    psv = ps[0:C, 0:512].rearrange("p (r c) -> p r c", c=256)
                    nc.tensor.matmul(psv, wsc_sb[0:2 * C, 0:C],
                                     ap2d(xc2, 2 * C, (y0 + 1) * PH + 1, 2),
                                     start=True, stop=True)
                    so = scoutp.tile([C, 512], f32, tag="so", name="scot")
                    nc.scalar.activation(so[0:C, 0:512], ps[0:C, 0:512],
                                         AF.Identity, bias=bsc_sb[0:C, 0:1],
                                         scale=1.0)
                    nc.sync.dma_start(
                        sc_d[0:C, y0:y0 + 2, :],
                        so[0:C, 0:512].rearrange("p (r c) -> p r c", c=256))

            # ---- AR2: u1 stats -------------------------------------
            sumsq(u1_pre, 16)
            stat_pack(cc_in[1], [u1_cols[0:C, 0:64], sq_cols[0:C, 16:24]])
            nc.gpsimd.collective_compute(
                "AllReduce", ALU.add, replica_groups=RG,
                ins=[cc_in[1][0:1, 0:256]], outs=[cc_out[1][0:1, 0:256]])
            inv_u1, nb_u1 = norm_coeffs(cc_out[1], 0, "u1")

            # ========== u normalize + u2 conv =======================
            with tc.tile_pool(name="u2inp", bufs=1, side="right") as u2inp:
                u2in = u2inp.tile([2 * C, PADF], f16, tag="u2in", name="u2in")
                nc.vector.memset(u2in[0:C, 0:259], 0.0)
                nc.vector.memset(u2in[0:C, 131 * PH:132 * PH + 8], 0.0)
                nc.vector.memset(
                    u2in[0:C, 515:515 + 130 * PH].rearrange(
                        "p (r k) -> p r k", k=PH)[:, :, 0:2], 0.0)
                uv = u2in[0:C, PH + 1:PH + 1 + EXT * PH].rearrange(
                    "p (r c) -> p r c", c=PH)[:, :, 0:256]
                nc.scalar.activation(
                    uv, u1_pre[0:C, 0:EXT * 256].rearrange(
                        "p (r c) -> p r c", c=256),
                    AF.Lrelu, bias=nb_u1[0:C, 0:1], scale=inv_u1[0:C, 0:1],
                    alpha=NEG)
                u1_es.close()
                u2_es = ExitStack()
                u2pool = u2_es.enter_context(tc.tile_pool(name="u2pool",
                                                          bufs=1))
                u2_pre = u2pool.tile([C, NT_OUT * 512], f16, tag="u2_pre",
                                     name="u2_pre")
                nc.sync.dma_start(u2in[C:2 * C, 0:132 * PH],
                                  u2in[0:C, 1:132 * PH + 1])
                with tc.tile_pool(name="u2ps", bufs=4, space="PSUM") as u2ps:
                    for t in range(NT_OUT):
                        y0 = 2 * t
                        ps = u2ps.tile([C, 512], f32, tag="ps", name="u2pst")
                        psv = ps[0:C, 0:512].rearrange("p (r c) -> p r c", c=256)
                        mms = []
                        for pi, dy in enumerate((-1, 0, 1)):
                            mms.append((wu2p_sb[0:2 * C, pi * C:(pi + 1) * C],
                                        ap2d(u2in, 2 * C, (y0 + dy + 1) * PH, 2)))
                        for si, dy in enumerate((-1, 0, 1)):
                            mms.append((wu2s_sb[0:C, si * C:(si + 1) * C],
                                        ap2d(u2in, C, (y0 + dy + 1) * PH + 2, 2)))
                        for i, (lh, rh) in enumerate(mms):
                            nc.tensor.matmul(psv, lh, rh, start=(i == 0),
                                             stop=(i == len(mms) - 1))
                        nc.scalar.activation(
                            u2_pre[0:C, t * 512:(t + 1) * 512], ps[0:C, 0:512],
                            AF.Identity, bias=bu2_sb[0:C, 0:1], scale=1.0,
                            accum_out=u2_cols[0:C, t:t + 1])

            # ---- AR3: u2 stats -------------------------------------
            sumsq(u2_pre, 24)
            stat_pack(cc_in[2], [u2_cols[0:C, 0:64], sq_cols[0:C, 24:32]])
            nc.gpsimd.collective_compute(
                "AllReduce", ALU.add, replica_groups=RG,
                ins=[cc_in[2][0:1, 0:256]], outs=[cc_out[2][0:1, 0:256]])
            inv_u2, nb_u2 = norm_coeffs(cc_out[2], 0, "u2")

            # ========== feat = lrelu(norm(u2)) + sc; maxpool ========
            sc_flat = sc_d.rearrange("c h w -> c (h w)")
            with tc.tile_pool(name="featp", bufs=2) as featp:
                for t in range(8):
                    sl = slice(t * 4096, (t + 1) * 4096)
                    ft = featp.tile([C, 4096], f32, tag="ft", name="ft")
                    nc.scalar.activation(ft[0:C, 0:4096], u2_pre[0:C, sl],
                                         AF.Lrelu, bias=nb_u2[0:C, 0:1],
                                         scale=inv_u2[0:C, 0:1], alpha=NEG)
                    sct = featp.tile([C, 4096], f32, tag="sct", name="sct")
                    nc.sync.dma_start(sct[0:C, 0:4096], sc_flat[0:C, sl])
                    nc.vector.tensor_add(ft[0:C, 0:4096], ft[0:C, 0:4096],
                                         sct[0:C, 0:4096])
                    nc.sync.dma_start(
                        feat_o[0:C, 16 * t:16 * (t + 1), :],
                        ft[0:C, 0:4096].rearrange("p (r c) -> p r c", c=256))
                    fv = ft[0:C, 0:4096].rearrange("p (r two c) -> p r two c",
                                                   two=2, c=256)
                    rm = featp.tile([C, 2048], f32, tag="rm", name="rm")
                    rmv = rm[0:C, 0:2048].rearrange("p (r c) -> p r c", c=256)
                    nc.vector.tensor_tensor(rmv, fv[:, :, 0, :], fv[:, :, 1, :],
                                            op=ALU.max)
                    rv = rm[0:C, 0:2048].rearrange("p (r c two) -> p r c two",
                                                   c=128, two=2)
                    pm = featp.tile([C, 1024], f32, tag="pm", name="pm")
                    pmv = pm[0:C, 0:1024].rearrange("p (r c) -> p r c", c=128)
                    nc.vector.tensor_tensor(pmv, rv[:, :, :, 0], rv[:, :, :, 1],
                                            op=ALU.max)
                    nc.sync.dma_start(
                        pooled_o[0:C, 8 * t:8 * (t + 1), :],
                        pm[0:C, 0:1024].rearrange("p (r c) -> p r c", c=128))
            u2_es.close()

    return nc


def _get_nc():
    if 'nc' not in _CACHE:
        _CACHE['nc'] = _build_nc()
    return _CACHE['nc']


# ----------------------------------------------------------------------
# entry point
# ----------------------------------------------------------------------

def kernel(**inputs):
    from concourse.bass_utils import run_bass_kernel_spmd

    per_core = host_prep(inputs)
    nc = _get_nc()
    res = run_bass_kernel_spmd(nc, per_core, core_ids=list(range(8)))

    feat = np.zeros((4, C, 256, 256), np.float32)
    pooled = np.zeros((4, C, 128, 128), np.float32)
    for s in range(4):
        r0 = res.results[2 * s]
        r1 = res.results[2 * s + 1]
        feat[s, :, 0:128, :] = r0['feat']
        feat[s, :, 128:256, :] = r1['feat'][:, ::-1, :]
        pooled[s, :, 0:64, :] = r0['pooled']
        pooled[s, :, 64:128, :] = r1['pooled'][:, ::-1, :]
    return feat, pooled
